# revision 20
# baseline (speedup 1.0000x reference)
"""Multi-head Hyena FFT long conv with fused gating — TRN2 Bass kernel.

Problem: nn_MultiHeadHyenaConv (B=2, D=1024, L=2048, num_heads=8, H=128
filter channels).  Reference semantics:
    kv[b,h,d1,d2,l] = v[b,h,d1,l] * x2[b,h,d2,l]
    y = causal_conv_l(kv, k[h]) + bias[h,d1] * kv
    out[b,h,d2,l]   = sum_d1 y[b,h,d1,d2,l] * x1[b,h,d1,l]

Strategy (per the sharding hint): tensor-parallel over the H=128 head
channels -> 8 cores x 16 heads, both batches per core, zero cross-core
communication.  Inputs are sliced host-side, outputs concatenated.

Per-core kernel: the causal conv is a lower-triangular Toeplitz matmul
done on the TensorEngine in fp16 (1 col/cycle) with fp32 PSUM
accumulation:
  - A "shifted kernel image" KS[p, x] = k[x - p] is built with one DMA
    per head from a host-side row-replicated staging array (all-positive
    flat strides; this walrus rejects negative AP steps).  Every 128x128
    Toeplitz block for diagonal offset j is the column slice
    KS[:, 128j:128j+128], reused as the matmul stationary across all
    output tiles and batches.
  - Inputs (and host-precomputed x1*bias) are loaded pre-transposed via
    xbar DMA-transpose into [lam, (b, lc, d)] fp16 layout.
  - KV[m, (b,d1,d2)] tiles form on the VectorEngine; conv matmuls are
    coalesced per psum bank (N up to 512 moving columns each).
  - PSUM accumulates over mi per li tile (bank-granular start/stop:
    start only on the first matmul touching a bank, per-element
    first-touch overwrites).
  - ScalarEngine drains PSUM to fp16; VectorEngine applies the x1 gate
    (pre-replicated over d2 by ScalarE) and reduces over d1 in-place.
  - The bias skip term is x2 * sum_d1 bias*x1*v, fused into the final add.
  - Outputs transpose back (PE) to natural [d, l] fp32; one large DMA
    out per batch.

Measured (8 cores, axon): max scale-relative error 8.6e-4 vs fp32
reference; steady-state main loop ~160 us/core (PE-bound: ~117 us conv
stream + ~35 us unoverlapped LDWEIGHTS — walrus ldw-opt is broken in
this toolchain, so every matmul reloads its stationary).
"""
import os
import sys

import numpy as np

for _p in ("/opt/trn_rl_repo", "/root/.axon_site/_ro/trn_rl_repo"):
    if os.path.isdir(_p) and _p not in sys.path:
        sys.path.insert(0, _p)

import concourse.bass as bass
import concourse.mybir as mybir
from concourse.bass_utils import run_bass_kernel_spmd
from concourse.tile import TileContext

f16, f32 = mybir.dt.float16, mybir.dt.float32

# --- workaround: this walrus build accepts at most ONE sem wait per
# instruction (TPB_EVENTS has a single wait slot) and refuses to split
# them itself for DMA/CTRL opcodes.  Post-process the BIR json: move
# extra waits onto single-wait NoOps inserted just before, same engine.
import json as _json

import concourse.bass_utils as _bu
import concourse.bass2jax as _b2j


def _split_multiwaits(bir_json):
    j = _json.loads(bir_json)
    changed = False
    for fn in j.get("functions", []):
        for bb in fn.get("blocks", []):
            insts = bb.get("instructions", [])
            out, ctr = [], 0
            for ins in insts:
                si = ins.get("sync_info")
                waits = (si or {}).get("on_wait") or []
                if len(waits) > 1:
                    changed = True
                    for w in waits[:-1]:
                        out.append({
                            "name": f"{ins['name']}-wsplit{ctr}",
                            "opcode": "NoOp",
                            "engine": ins["engine"],
                            "ins": [], "outs": [],
                            "debug": ins.get("debug", 0),
                            "sync_info": {"on_wait": [w], "on_update": []},
                        })
                        ctr += 1
                    si["on_wait"] = [waits[-1]]
                out.append(ins)
            if ctr:
                bb["instructions"] = out
    if not changed:
        return bir_json
    return _json.dumps(j).encode()




# --- LDWEIGHTS dedup: walrus (ldw-opt is broken/disabled) re-emits an
# identical LDWEIGHTS before every matmul.  Consecutive byte-identical
# LDW records (same weights AP, same idempotent sem wait) reload weights
# the PE array already holds — overwrite them with NOPs in the packed
# NEFF (same 64B record size, no address shifts).
import io as _io
import tarfile as _tarfile
import tempfile as _tempfile

from concourse import neff as _neff


def _dedup_ldweights_in_neff(neff_path):
    with open(neff_path, "rb") as f:
        old_header = f.read(1024)
        tar_data = f.read()
    with _tempfile.TemporaryDirectory() as d:
        with _tarfile.open(fileobj=_io.BytesIO(tar_data), mode="r") as t:
            t.extractall(d)
        pe_path = os.path.join(d, "sg00", "PE0.bin")
        if not os.path.exists(pe_path):
            return
        data = bytearray(open(pe_path, "rb").read())
        nop = bytes([0xA4, 0x10, 0, 0]) + bytes(60)
        last_key, n = None, 0
        for i in range(0, len(data), 64):
            rec = bytes(data[i:i + 64])
            if rec[0] == 0x01:  # LDWEIGHTS
                key = rec[:2] + rec[4:]
                if key == last_key:
                    data[i:i + 64] = nop
                    n += 1
                else:
                    last_key = key
            elif rec[0] != 0xA4:  # any non-NOP invalidates tracking
                if rec[0] != 0x02:  # MATMUL keeps array weights
                    last_key = None
        if not n:
            return
        open(pe_path, "wb").write(bytes(data))
        buf = _io.BytesIO()
        with _tarfile.open(fileobj=buf, mode="w") as t:
            t.add(d, arcname=".", filter=_b2j._reset_tarinfo)
        new_data = buf.getvalue()
        new_header = _neff.make_deterministic_neff_header(
            old_neff_header=old_header, new_neff_data=new_data)
    with open(neff_path, "wb") as f:
        f.write(new_header + new_data)


_orig_compile_bir_kernel = _bu.compile_bir_kernel


def _compile_bir_kernel_splitwaits(bir_json, tmpdir, neff_name="file.neff"):
    neff_path = _orig_compile_bir_kernel(_split_multiwaits(bir_json), tmpdir, neff_name)
    try:
        _dedup_ldweights_in_neff(neff_path)
    except Exception as e:  # non-fatal: unpatched NEFF is still correct
        print(f"ldw-dedup skipped: {e!r}")
    return neff_path


if _bu.compile_bir_kernel is not _compile_bir_kernel_splitwaits:
    _bu.compile_bir_kernel = _compile_bir_kernel_splitwaits
    _b2j.compile_bir_kernel = _compile_bir_kernel_splitwaits

MUL, ADD = mybir.AluOpType.mult, mybir.AluOpType.add

B, D, L, NH = 2, 1024, 2048, 8
N_CORES = 8
H = D // NH              # 128 filter channels
HL = H // N_CORES        # 16 heads per core
DL = HL * NH             # 128 data channels per core
NT = L // 128            # 16 sequence tiles
KP = L + 127             # padded kernel row length


def _build(nc: bass.Bass, heads: int = HL, conv: bool = True, gate: bool = True, reps: int = 1, coalesce: bool = True, direct_gate: bool = False, dma_tr: bool = True):
    v16 = nc.dram_tensor("v16", [B, DL, L], f16, kind="ExternalInput")
    x1_16 = nc.dram_tensor("x1_16", [B, DL, L], f16, kind="ExternalInput")
    x2_16 = nc.dram_tensor("x2_16", [B, DL, L], f16, kind="ExternalInput")
    x1b16 = nc.dram_tensor("x1b16", [B, DL, L], f16, kind="ExternalInput")
    kstag16 = nc.dram_tensor("kstag16", [HL, 128, KP], f16, kind="ExternalInput")
    ident_h = nc.dram_tensor("ident_h", [DL, DL], f16, kind="ExternalInput")
    identb_h = nc.dram_tensor("identb_h", [DL, DL], f16, kind="ExternalInput")
    ident_s = nc.dram_tensor("ident_s", [128, 128], f32, kind="ExternalInput")
    out_d = nc.dram_tensor("out", [B, DL, L], f32, kind="ExternalOutput")

    with TileContext(nc) as tc:
        with tc.tile_pool(name="const", bufs=1) as constp, \
             tc.tile_pool(name="persist", bufs=1) as persist:
            idh = constp.tile([DL, DL], f16)
            idbh = constp.tile([DL, DL], f16)
            ids = constp.tile([128, 128], f32)
            nc.sync.dma_start(out=idh[:], in_=ident_h[:])
            nc.sync.dma_start(out=idbh[:], in_=identb_h[:])
            nc.sync.dma_start(out=ids[:], in_=ident_s[:])

            # persistent transposed inputs: [128 lam, (b, lc, d)] fp16
            vT = persist.tile([128, B * NT * DL], f16)
            x1T = persist.tile([128, B * NT * DL], f16)
            x1bT = persist.tile([128, B * NT * DL], f16)
            x2T = persist.tile([128, B * NT * DL], f16)
            WT = persist.tile([128, B * NT * HL], f32)
            outT32 = persist.tile([128, B * NT * DL], f32)
            out_nat = persist.tile([DL, B * L], f32)

            # ---- transposed input loads ----
            if dma_tr:
                # xbar DMA transpose straight from HBM; x1*bias precomputed host-side
                for b in range(B):
                    for (src, dstT) in ((v16, vT), (x2_16, x2T), (x1_16, x1T), (x1b16, x1bT)):
                        for lc in range(NT):
                            col = (b * NT + lc) * DL
                            nc.sync.dma_start_transpose(
                                out=dstT[:, col:col + DL],
                                in_=src[b][:, lc * 128:(lc + 1) * 128])
            else:
              with tc.tile_pool(name="nat", bufs=3) as natp, \
                 tc.tile_pool(name="pst", bufs=3, space="PSUM") as pst:
                for b in range(B):
                    for (src, dstT) in ((v16, vT), (x2_16, x2T), (x1_16, x1T)):
                        nat = natp.tile([DL, L], f16, tag="nat")
                        nc.sync.dma_start(out=nat[:], in_=src[b])
                        for lc in range(NT):
                            ps = pst.tile([128, DL], f16, tag="ps")
                            nc.tensor.transpose(ps[:], nat[:, lc * 128:(lc + 1) * 128], idh[:])
                            col = (b * NT + lc) * DL
                            nc.scalar.copy(out=dstT[:, col:col + DL], in_=ps[:])
                            if dstT is x1T:  # bias-scaled transpose via plain matmul
                                ps2 = pst.tile([128, DL], f32, tag="ps2")
                                nc.tensor.matmul(ps2[:], nat[:, lc * 128:(lc + 1) * 128], idbh[:],
                                                 start=True, stop=True)
                                nc.scalar.copy(out=x1bT[:, col:col + DL], in_=ps2[:])

            # ---- W term: WT[lam, (b, lc, h)] = sum_d1 bias*x1*v ----
            with tc.tile_pool(name="wtmp", bufs=1) as wtmp:
                t = wtmp.tile([128, B * NT * DL], f16)
                nc.vector.tensor_tensor(out=t[:], in0=x1bT[:], in1=vT[:], op=MUL)
                n_grp = B * NT * HL
                for width in (4, 2):
                    a0 = bass.AP(t[:].tensor, t[:].offset, [[B * NT * DL, 128], [NH, n_grp], [1, width]])
                    a1 = bass.AP(t[:].tensor, t[:].offset + width, [[B * NT * DL, 128], [NH, n_grp], [1, width]])
                    nc.vector.tensor_tensor(out=a0, in0=a0, in1=a1, op=ADD)
                a0 = bass.AP(t[:].tensor, t[:].offset, [[B * NT * DL, 128], [NH, n_grp]])
                a1 = bass.AP(t[:].tensor, t[:].offset + 1, [[B * NT * DL, 128], [NH, n_grp]])
                nc.vector.tensor_tensor(out=WT[:], in0=a0, in1=a1, op=ADD)

            # ---- main per-head loop ----
            with tc.tile_pool(name="ks", bufs=HL) as ksp, \
                 tc.tile_pool(name="kv", bufs=3) as kvp, \
                 tc.tile_pool(name="x1r", bufs=3) as x1rp, \
                 tc.tile_pool(name="y16", bufs=3) as y16p, \
                 tc.tile_pool(name="g", bufs=3) as gp, \
                 tc.tile_pool(name="psc", bufs=2, space="PSUM") as psc:
              if True:
                import contextlib
                rep_ctx = tc.For_i(0, reps, 1) if reps > 1 else contextlib.nullcontext()
                with rep_ctx:
                 if True:
                    front = {}
                    # prefetch all KS images up front: the 512KB strided DMA
                    # (~5us each) otherwise serializes per head; alternate the
                    # two HWDGE rings (SP / ACT) for parallel drain
                    KS_all = {}
                    for h in range(heads):
                        KS = ksp.tile([128, L], f16, tag="ks")
                        ksrc = bass.AP(kstag16[:].tensor, h * 128 * KP + 127,
                                       [[KP - 1, 128], [1, L]])
                        eng = nc.sync if h % 2 == 0 else nc.scalar
                        eng.dma_start(out=KS[:], in_=ksrc)
                        KS_all[h] = KS

                    def emit_front(h):
                        KS = KS_all[h]
                        KV = kvp.tile([128, NT * 128], f16, tag="kv")
                        for b in range(B):
                            i0 = bass.AP(vT[:].tensor, vT[:].offset + b * NT * DL + h * NH,
                                         [[B * NT * DL, 128], [DL, NT], [1, NH], [0, NH]])
                            i1 = bass.AP(x2T[:].tensor, x2T[:].offset + b * NT * DL + h * NH,
                                         [[B * NT * DL, 128], [DL, NT], [0, NH], [1, NH]])
                            o = bass.AP(KV[:].tensor, KV[:].offset + b * NH * NH,
                                        [[NT * 128, 128], [128, NT], [NH, NH], [1, NH]])
                            nc.vector.tensor_tensor(out=o, in0=i0, in1=i1, op=MUL)
                        X1R = x1rp.tile([128, NT * 128], f16, tag="x1r")
                        for b in range(B):
                            i0 = bass.AP(x1T[:].tensor, x1T[:].offset + b * NT * DL + h * NH,
                                         [[B * NT * DL, 128], [DL, NT], [1, NH], [0, NH]])
                            o = bass.AP(X1R[:].tensor, X1R[:].offset + b * NH * NH,
                                        [[NT * 128, 128], [128, NT], [NH, NH], [1, NH]])
                            nc.scalar.copy(out=o, in_=i0)
                        front[h] = (KS, KV, X1R)

                    def emit_conv(h):
                        KS, KV, X1R = front[h]
                        psum = psc.tile([128, NT * 128], f32, tag="ps")
                        NJ = NT if conv is True else int(conv)
                        for j in range(NJ):
                            lhsT = KS[:, j * 128:(j + 1) * 128]
                            if coalesce:
                                for bk in range(NT // 4):
                                    lo = max(j, 4 * bk)
                                    hi = 4 * bk + 3
                                    if lo > hi:
                                        continue
                                    cnt = hi - lo + 1
                                    mi0 = lo - j
                                    nc.tensor.matmul(
                                        psum[:, lo * 128:(hi + 1) * 128], lhsT,
                                        KV[:, mi0 * 128:(mi0 + cnt) * 128],
                                        start=(j == 0), stop=(j == min(hi, NJ - 1)))
                            else:
                                for li in range(j, NT):
                                    mi = li - j
                                    nc.tensor.matmul(
                                        psum[:, li * 128:(li + 1) * 128], lhsT,
                                        KV[:, mi * 128:(mi + 1) * 128],
                                        start=(j == 0 and li % 4 == 0),
                                        stop=(j == li and li % 4 == 3))
                        return psum

                    def emit_back(h, psum):
                        KS, KV, X1R = front.pop(h)
                        g = gp.tile([128, NT * 128], f16, tag="g")
                        if direct_gate:
                            for bank in range(NT // 4 if gate else 0):
                                sl = slice(bank * 512, bank * 512 + 512)
                                nc.vector.tensor_tensor(out=g[:, sl], in0=psum[:, sl], in1=X1R[:, sl], op=MUL)
                        else:
                            y16 = y16p.tile([128, NT * 128], f16, tag="y16")
                            for bank in range(NT // 4 if gate else 0):
                                sl = slice(bank * 512, bank * 512 + 512)
                                nc.scalar.copy(out=y16[:, sl], in_=psum[:, sl])
                                nc.vector.tensor_tensor(out=g[:, sl], in0=y16[:, sl], in1=X1R[:, sl], op=MUL)
                        for b in range(B):
                            for width in (4, 2, 1):
                                a0 = bass.AP(g[:].tensor, g[:].offset + b * NH * NH,
                                             [[NT * 128, 128], [128, NT], [NH, width], [1, NH]])
                                a1 = bass.AP(g[:].tensor, g[:].offset + b * NH * NH + width * NH,
                                             [[NT * 128, 128], [128, NT], [NH, width], [1, NH]])
                                nc.vector.tensor_tensor(out=a0, in0=a0, in1=a1, op=ADD)
                        o32 = bass.AP(outT32[:].tensor, outT32[:].offset + h * NH,
                                      [[B * NT * DL, 128], [NT * DL, B], [DL, NT], [1, NH]])
                        i_x2 = bass.AP(x2T[:].tensor, x2T[:].offset + h * NH,
                                       [[B * NT * DL, 128], [NT * DL, B], [DL, NT], [1, NH]])
                        i_wt = bass.AP(WT[:].tensor, WT[:].offset + h,
                                       [[B * NT * HL, 128], [NT * HL, B], [HL, NT], [0, NH]])
                        nc.vector.tensor_tensor(out=o32, in0=i_x2, in1=i_wt, op=MUL)
                        i_g = bass.AP(g[:].tensor, g[:].offset,
                                      [[NT * 128, 128], [NH * NH, B], [128, NT], [1, NH]])
                        nc.vector.tensor_tensor(out=o32, in0=o32, in1=i_g, op=ADD)

                    # software pipeline: next head's KS/KV/X1R are emitted
                    # before this head's gate stage so DVE produces KV(h+1)
                    # ahead of its own gate tail and PE never waits
                    if heads > 0:
                        emit_front(0)
                    for h in range(heads):
                        psum = emit_conv(h)
                        if h + 1 < heads:
                            emit_front(h + 1)
                        emit_back(h, psum)

            # ---- transpose back to natural + store ----
            with tc.tile_pool(name="pso", bufs=4, space="PSUM") as pso:
                for b in range(B):
                    for lc in range(NT):
                        ps = pso.tile([DL, 128], f32, tag="pso")
                        col = (b * NT + lc) * DL
                        nc.tensor.transpose(ps[:], outT32[:, col:col + DL], ids[:])
                        nc.scalar.copy(out=out_nat[:, b * L + lc * 128: b * L + (lc + 1) * 128],
                                       in_=ps[:])
                for b in range(B):
                    nc.sync.dma_start(out=out_d[b], in_=out_nat[:, b * L:(b + 1) * L])
    return nc


_NC_CACHE = {}


def _get_nc():
    if "nc" not in _NC_CACHE:
        nc = bass.Bass()
        _build(nc)
        _NC_CACHE["nc"] = nc
    return _NC_CACHE["nc"]


def make_in_maps(v, k, bias, x1, x2):
    v16 = np.asarray(v, np.float32).astype(np.float16)
    x1_16 = np.asarray(x1, np.float32).astype(np.float16)
    x2_16 = np.asarray(x2, np.float32).astype(np.float16)
    k32 = np.asarray(k, np.float32)
    bias16 = np.asarray(bias, np.float32).astype(np.float16)
    kpad = np.zeros((H, KP), np.float16)
    kpad[:, 127:] = k32.astype(np.float16)
    ident_h = np.eye(DL, dtype=np.float16)
    ident_s = np.eye(128, dtype=np.float32)
    in_maps = []
    for c in range(N_CORES):
        dsl = slice(c * DL, (c + 1) * DL)
        hsl = slice(c * HL, (c + 1) * HL)
        in_maps.append({
            "v16": np.ascontiguousarray(v16[:, dsl, :]),
            "x1_16": np.ascontiguousarray(x1_16[:, dsl, :]),
            "x1b16": np.ascontiguousarray(
                (np.asarray(x1, np.float32)[:, dsl, :]
                 * np.asarray(bias, np.float32)[dsl][None, :, None]).astype(np.float16)),
            "x2_16": np.ascontiguousarray(x2_16[:, dsl, :]),
            "kstag16": np.ascontiguousarray(np.broadcast_to(
                kpad[hsl][:, None, :], (HL, 128, KP))),
            "ident_h": ident_h,
            "identb_h": np.diag(bias16[dsl]).astype(np.float16),
            "ident_s": ident_s,
        })
    return in_maps


def kernel(v, k, bias, x1, x2, num_heads):
    assert int(num_heads) == NH
    in_maps = make_in_maps(v, k, bias, x1, x2)
    res = run_bass_kernel_spmd(_get_nc(), in_maps, list(range(N_CORES)))
    out = np.concatenate([res.results[c]["out"] for c in range(N_CORES)], axis=1)
    return out.astype(np.float32)
